# revision 9
# baseline (speedup 1.0000x reference)
"""Multi-head attention (B=4, S=2048, D=768, H=12) on 8 trn2 NeuronCores.

Sharding: 48 (batch, head) slices over 8 cores; core i handles batch i//2 and
heads 6*(i%2) .. 6*(i%2)+6 (tensor-parallel columns of Wq/Wk/Wv, rows of Wo).
Host sums the two partial output projections per batch (the "all-reduce after
W_o") and adds the bias terms.

Device-side layout: everything is computed with the contraction dim on SBUF
partitions so no on-chip transposes are needed:
  - host ships x[b].T (D, S) and W*.T slices
  - QT/KT are (384, S) with head-dim on partitions
  - scores are built transposed: ST[k, q] = sum_d KT[d,k] QT[d,q]
  - exp via ScalarE (scale=1/sqrt(dk) folded in); softmax denominators come
    from a ones-column appended to V during the context matmul (row 64 of the
    context PSUM accumulator is the per-query colsum of exp scores)
  - reciprocal of the colsum is computed on a (16,128) partition-major view
    (cheap on DVE) and replicated to all 128 partitions with a stride-0 DMA
  - attention weights output is written transposed [h][k][q]; the host
    returns a transposed view so no device transpose is ever done.
"""

import sys
import functools
from contextlib import ExitStack

import numpy as np

if "/opt/trn_rl_repo" not in sys.path:
    sys.path.insert(0, "/opt/trn_rl_repo")

import ml_dtypes  # noqa: E402

import concourse.bass as bass  # noqa: E402
import concourse.mybir as mybir  # noqa: E402
import concourse.tile as tile  # noqa: E402
from concourse import bacc  # noqa: E402
from concourse.bass import ts, ds  # noqa: E402

BF = mybir.dt.bfloat16
F32 = mybir.dt.float32
BF_NP = ml_dtypes.bfloat16

D = 768          # model dim
S = 2048         # sequence length
NH = 12          # total heads
DK = 64          # head dim
B = 4            # batch
HPC = 6          # heads per core
JW = HPC * DK    # 384: local head width per core
NCORES = 8
SCALE = 1.0 / 8.0  # 1/sqrt(DK)
ET_BUFS = 22     # eT tile pool depth (each tile 128x2048 bf16 = 512KB)


def _body(ctx, tc, xT, wqT, wkT, wvT, woT, bqk, w_out, y_out):
    nc = tc.nc
    Exp = mybir.ActivationFunctionType.Exp
    mult = mybir.AluOpType.mult

    consts = ctx.enter_context(tc.tile_pool(name="consts", bufs=1))
    wpool = ctx.enter_context(tc.tile_pool(name="wpool", bufs=1))
    qk_pool = ctx.enter_context(tc.tile_pool(name="qk", bufs=1))
    vpool = ctx.enter_context(tc.tile_pool(name="vaug", bufs=1))
    rp_pool = ctx.enter_context(tc.tile_pool(name="rp", bufs=2))
    cr_pool = ctx.enter_context(tc.tile_pool(name="ctxraw", bufs=1))
    cs_pool = ctx.enter_context(tc.tile_pool(name="csmall", bufs=2))
    ctx_pool = ctx.enter_context(tc.tile_pool(name="ctxsb", bufs=1))
    outp = ctx.enter_context(tc.tile_pool(name="outsb", bufs=3))
    dram = ctx.enter_context(tc.tile_pool(name="dram", bufs=2, space="DRAM"))
    # 3x (128,1024) scores tiles (6 banks) keep the PE ahead of ACT's exp
    # drain; 2x (128,512) work tiles (2 banks) for qkv/ctx/proj accumulation.
    ps_scores = ctx.enter_context(tc.tile_pool(name="ps_s", bufs=3, space="PSUM"))
    ps_work = ctx.enter_context(tc.tile_pool(name="ps_w", bufs=2, space="PSUM"))

    # --- load weights/biases ---
    wq_sb = wpool.tile([128, 6, JW], BF, tag="wq")
    wk_sb = wpool.tile([128, 6, JW], BF, tag="wk")
    wv_sb = wpool.tile([128, 6, JW], BF, tag="wv")
    wo_sb = wpool.tile([128, 3, D], BF, tag="wo")
    bqk_sb = consts.tile([128, 6], F32, tag="bqk")
    nc.sync.dma_start(wq_sb[:], wqT.rearrange("(o p) j -> p o j", p=128))
    nc.sync.dma_start(wk_sb[:], wkT.rearrange("(o p) j -> p o j", p=128))
    nc.sync.dma_start(wv_sb[:], wvT.rearrange("(o p) j -> p o j", p=128))
    nc.sync.dma_start(wo_sb[:], woT.rearrange("(o p) e -> p o e", p=128))
    nc.sync.dma_start(bqk_sb[:], bqk[:, :])

    qt_sb = qk_pool.tile([128, 3, S], BF, tag="qt")
    kt_sb = qk_pool.tile([128, 3, S], BF, tag="kt")
    v_sb = vpool.tile([128, 16, HPC * 65], BF, tag="v")  # V with ones col per head

    # --- phase 1: QKV projections (x lives in its own pool, freed after) ---
    with tc.tile_pool(name="xpool", bufs=1) as xpool:
        x_sb = xpool.tile([128, 6, S], BF, tag="x")
        nc.sync.dma_start(x_sb[:], xT.rearrange("(o p) s -> p o s", p=128))

        for w_sb, dest, bcol0 in ((wq_sb, qt_sb, 0), (wk_sb, kt_sb, 3)):
            for jo in range(3):
                for st in range(4):
                    ps = ps_work.tile([128, 512], F32, tag="psw")
                    for o in range(6):
                        nc.tensor.matmul(
                            ps[:],
                            lhsT=w_sb[:, o, ts(jo, 128)],
                            rhs=x_sb[:, o, ts(st, 512)],
                            start=(o == 0),
                            stop=(o == 5),
                        )
                    # evacuate + add per-partition bias (torch Linear bias)
                    nc.vector.tensor_tensor(
                        dest[:, jo, ts(st, 512)], ps[:],
                        bqk_sb[:, bcol0 + jo : bcol0 + jo + 1]
                        .to_broadcast([128, 512]),
                        mybir.AluOpType.add,
                    )

        # ones columns for the softmax-denominator trick
        v4 = v_sb.rearrange("p k (h c) -> p k h c", c=65)
        nc.gpsimd.memset(v4[:, :, :, 64:65], 1.0)
        for kt in range(16):
            ps = ps_work.tile([128, 512], F32, tag="psw")
            for o in range(6):
                nc.tensor.matmul(
                    ps[:, 0:JW],
                    lhsT=x_sb[:, o, ts(kt, 128)],
                    rhs=wv_sb[:, o, :],
                    start=(o == 0),
                    stop=(o == 5),
                )
            # V bias is folded out: context of unnormalized weights plus the
            # colsum row lets the host add bv @ Wo.T once per batch.
            nc.vector.tensor_copy(
                out=v4[:, kt, :, 0:64],
                in_=ps[:, 0:JW].rearrange("p (h c) -> p h c", c=64),
            )

    # --- phase 2: attention, software-pipelined over heads ---
    ctxT_sb = ctx_pool.tile([128, 3, S], BF, tag="ctxT")
    et_tiles = {}
    craw = None
    rp_prev = None
    for ph in range(7):
        h = ph if ph < 6 else None        # head doing scores+exp this phase
        hc = ph - 1 if ph >= 1 else None  # head doing context this phase

        if hc is not None:
            craw = cr_pool.tile([65, S], F32, tag="craw")
            cs_dram = dram.tile([S], F32, tag="cs")
            rb_dram = dram.tile([S], BF, tag="rb")
            colsum2 = cs_pool.tile([16, 128], F32, tag="cs2")
            recip2 = cs_pool.tile([16, 128], F32, tag="rc2")
            recipb = cs_pool.tile([16, 128], BF, tag="rcb")
            rp = rp_pool.tile([128, S], BF, tag="rp")
            ps_ctx = None

        for kt in range(16):
            if h is not None:
                po, o = 64 * (h % 2), h // 2
                et = et_pool.tile([128, S], BF, tag="et")
                et_tiles[(h, kt)] = et
                for half in range(2):
                    ps_s = ps_scores.tile([128, 1024], F32, tag="pss")
                    for st in (2 * half, 2 * half + 1):
                        nc.tensor.matmul(
                            ps_s[:, ts(st - 2 * half, 512)],
                            lhsT=kt_sb[po : po + 64, o, ts(kt, 128)],
                            rhs=qt_sb[po : po + 64, o, ts(st, 512)],
                            start=True,
                            stop=True,
                        )
                    nc.scalar.activation(
                        et[:, ts(half, 1024)], ps_s[:], Exp, scale=SCALE
                    )

            if hc is not None:
                s, base = kt // 4, 4 * (kt % 4)
                if base == 0:
                    ps_ctx = ps_work.tile([128, 512], F32, tag="psw")
                for k2 in range(base, base + 4):
                    nc.tensor.matmul(
                        ps_ctx[0:65, :],
                        lhsT=v_sb[:, k2, ds(65 * hc, 65)],
                        rhs=et_tiles[(hc, k2)][:, ts(s, 512)],
                        start=(k2 == 0),
                        stop=(k2 == 15),
                    )
                if base == 12:
                    # strip finished: rows 0..63 = raw context.T, row 64 = colsum
                    nc.vector.tensor_copy(craw[:, ts(s, 512)], ps_ctx[0:65, :])
                    nc.sync.dma_start(cs_dram[ts(s, 512)], craw[64:65, ts(s, 512)])
                if kt == 15:
                    # colsum -> (16,128) partition-major -> 1/x -> bf16 ->
                    # replicate to 128 partitions via stride-0 DMA
                    nc.sync.dma_start(
                        colsum2[:], cs_dram.rearrange("(a b) -> a b", b=128)
                    )
                    nc.vector.reciprocal(recip2[:], colsum2[:])
                    nc.vector.tensor_copy(recipb[:], recip2[:])
                    nc.sync.dma_start(rb_dram[:], recipb[:])
                    nc.sync.dma_start(
                        rp[:],
                        rb_dram.rearrange("(o f) -> o f", o=1)
                        .to_broadcast([128, S]),
                    )

        if hc is not None:
            # normalize context rows of this head into the bf16 ctx buffer
            pc, oc = 64 * (hc % 2), hc // 2
            nc.vector.tensor_tensor(
                ctxT_sb[pc : pc + 64, oc, :], craw[0:64, :], rp[0:64, :], mult
            )
            # normalize exp-scores in place and ship them out
            for kt in range(16):
                et = et_tiles.pop((hc, kt))
                nc.vector.tensor_tensor(et[:], et[:], rp[:], mult)
                nc.sync.dma_start(w_out[hc, ts(kt, 128), :], et[:])
            rp_prev = rp

    # --- phase 3: output projection (partial; host sums across 2 cores) ---
    for st2 in range(16):
        ob = outp.tile([128, D], BF, tag="ob")
        for c0, cw in ((0, 512), (512, 256)):
            ps = ps_work.tile([128, 512], F32, tag="psw")
            for o in range(3):
                nc.tensor.matmul(
                    ps[:, 0:cw],
                    lhsT=ctxT_sb[:, o, ts(st2, 128)],
                    rhs=wo_sb[:, o, ds(c0, cw)],
                    start=(o == 0),
                    stop=(o == 2),
                )
            nc.scalar.copy(ob[:, ds(c0, cw)], ps[:, 0:cw])
        nc.sync.dma_start(y_out[ts(st2, 128), :], ob[:])


def build_bass(finalize=True):
    global et_pool
    nc = bacc.Bacc(None, target_bir_lowering=False, debug=False)
    xT = nc.declare_dram_parameter("xT", [D, S], BF, isOutput=False)
    wqT = nc.declare_dram_parameter("wqT", [D, JW], BF, isOutput=False)
    wkT = nc.declare_dram_parameter("wkT", [D, JW], BF, isOutput=False)
    wvT = nc.declare_dram_parameter("wvT", [D, JW], BF, isOutput=False)
    woT = nc.declare_dram_parameter("woT", [JW, D], BF, isOutput=False)
    bqk = nc.declare_dram_parameter("bqk", [128, 6], F32, isOutput=False)
    w_out = nc.declare_dram_parameter("w_out", [HPC, S, S], BF, isOutput=True)
    y_out = nc.declare_dram_parameter("y_out", [S, D], BF, isOutput=True)

    with tile.TileContext(nc) as tc:
        with ExitStack() as ctx:
            et_pool = ctx.enter_context(tc.tile_pool(name="eT", bufs=ET_BUFS))
            _body(ctx, tc, xT, wqT, wkT, wvT, woT, bqk, w_out, y_out)
    if finalize:
        nc.finalize()
    return nc


@functools.lru_cache(maxsize=1)
def _built():
    return build_bass()


def make_in_maps(x, Wq, bq, Wk, bk, Wv, bv, Wo, bo):
    in_maps = []
    for core in range(NCORES):
        b = core // 2
        hb = HPC * (core % 2)
        rows = slice(hb * DK, hb * DK + JW)
        bqk = np.empty((128, 6), np.float32)
        for jo in range(3):
            bqk[:, jo] = bq[rows][jo * 128 : (jo + 1) * 128]
            bqk[:, 3 + jo] = bk[rows][jo * 128 : (jo + 1) * 128]
        in_maps.append(
            dict(
                xT=np.ascontiguousarray(x[b].T).astype(BF_NP),
                wqT=np.ascontiguousarray(Wq[rows].T).astype(BF_NP),
                wkT=np.ascontiguousarray(Wk[rows].T).astype(BF_NP),
                wvT=np.ascontiguousarray(Wv[rows].T).astype(BF_NP),
                woT=np.ascontiguousarray(Wo[:, rows].T).astype(BF_NP),
                bqk=bqk,
            )
        )
    return in_maps


def assemble(results, Wq=None, bq=None, Wk=None, bk=None, Wv=None, bv=None,
             Wo=None, bo=None):
    """Gather per-core outputs into full (output, weights)."""
    wf = np.empty((B, NH, S, S), np.float32)  # [b, h, k, q] order
    y = np.empty((B, S, D), np.float32)
    const = (Wo @ bv + bo).astype(np.float32)  # bv @ Wo.T + bo
    for b in range(B):
        r0, r1 = results[2 * b], results[2 * b + 1]
        y[b] = (
            np.asarray(r0["y_out"]).astype(np.float32)
            + np.asarray(r1["y_out"]).astype(np.float32)
            + const
        )
        wf[b, 0:HPC] = np.asarray(r0["w_out"]).astype(np.float32)
        wf[b, HPC:NH] = np.asarray(r1["w_out"]).astype(np.float32)
    weights = wf.transpose(0, 1, 3, 2)  # view: [b, h, q, k]
    return y, weights


def kernel(x, Wq, bq, Wk, bk, Wv, bv, Wo, bo):
    from concourse.bass_utils import run_bass_kernel_spmd

    nc = _built()
    in_maps = make_in_maps(x, Wq, bq, Wk, bk, Wv, bv, Wo, bo)
    res = run_bass_kernel_spmd(nc, in_maps, core_ids=list(range(NCORES)))
    return assemble(res.results, Wq, bq, Wk, bk, Wv, bv, Wo, bo)


# revision 13
# speedup vs baseline: 1.0150x; 1.0150x over previous
"""Multi-head attention (B=4, S=2048, D=768, H=12) on 8 trn2 NeuronCores.

Sharding: 48 (batch, head) slices over 8 cores; core i handles batch i//2 and
heads 6*(i%2) .. 6*(i%2)+6 (tensor-parallel columns of Wq/Wk/Wv, rows of Wo).
Host sums the two partial output projections per batch (the "all-reduce after
W_o") and adds the bias terms.

Device-side layout: everything is computed with the contraction dim on SBUF
partitions so no on-chip transposes are needed:
  - host ships x[b].T (D, S) and W*.T slices
  - QT/KT are (384, S) with head-dim on partitions
  - scores are built transposed: ST[k, q] = sum_d KT[d,k] QT[d,q]
  - exp via ScalarE (scale=1/sqrt(dk) folded in); softmax denominators come
    from a ones-column appended to V during the context matmul (row 64 of the
    context PSUM accumulator is the per-query colsum of exp scores)
  - reciprocal of the colsum is computed on a partition-major view (cheap on
    DVE) and replicated to all 128 partitions with a stride-0 DMA
  - attention weights output is written transposed [h][k][q]; the host
    returns a transposed view so no device transpose is ever done.

The attention loop is a 3-stage software pipeline over (head, query-half)
units: phase p runs scores+exp for unit p, context matmuls for unit p-1, and
the softmax-normalize + DMA-out for unit p-2. Query-halving keeps the live
exp-score working set at ~2 generations x 16 x (128,1024)bf16 = 8MB so the
TensorEngine never starves on pool slots (starvation de-warms the PE clock).
"""

import sys
import functools
from contextlib import ExitStack

import numpy as np

if "/opt/trn_rl_repo" not in sys.path:
    sys.path.insert(0, "/opt/trn_rl_repo")

import ml_dtypes  # noqa: E402

import concourse.bass as bass  # noqa: E402
import concourse.mybir as mybir  # noqa: E402
import concourse.tile as tile  # noqa: E402
from concourse import bacc  # noqa: E402
from concourse.bass import ts, ds  # noqa: E402

BF = mybir.dt.bfloat16
F32 = mybir.dt.float32
BF_NP = ml_dtypes.bfloat16

D = 768          # model dim
S = 2048         # sequence length
NH = 12          # total heads
DK = 64          # head dim
B = 4            # batch
HPC = 6          # heads per core
JW = HPC * DK    # 384: local head width per core
NCORES = 8
SCALE = 1.0 / 8.0  # 1/sqrt(DK)
HQ = 1024        # query-half width
NP = 12          # pipeline units: HPC heads x 2 query halves
ET_BUFS = 36     # eT pool: (128,1024)bf16 tiles, ~2KB/partition each


def _body(ctx, tc, xT, wqT, wkT, wvT, woT, bqk, w_out, y_out):
    nc = tc.nc
    Exp = mybir.ActivationFunctionType.Exp
    mult = mybir.AluOpType.mult

    consts = ctx.enter_context(tc.tile_pool(name="consts", bufs=1))
    wpool = ctx.enter_context(tc.tile_pool(name="wpool", bufs=1))
    qk_pool = ctx.enter_context(tc.tile_pool(name="qk", bufs=1))
    vpool = ctx.enter_context(tc.tile_pool(name="vaug", bufs=1))
    ctx_pool = ctx.enter_context(tc.tile_pool(name="ctxsb", bufs=1))
    dram = ctx.enter_context(tc.tile_pool(name="dram", bufs=3, space="DRAM"))
    ps_scores = ctx.enter_context(tc.tile_pool(name="ps_s", bufs=3, space="PSUM"))
    ps_work = ctx.enter_context(tc.tile_pool(name="ps_w", bufs=2, space="PSUM"))

    wo_sb = wpool.tile([128, 3, D], BF, tag="wo")
    bqk_sb = consts.tile([128, 6], F32, tag="bqk")
    nc.sync.dma_start(wo_sb[:], woT.rearrange("(o p) e -> p o e", p=128))
    nc.sync.dma_start(bqk_sb[:], bqk[:, :])

    qt_sb = qk_pool.tile([128, 3, S], BF, tag="qt")
    kt_sb = qk_pool.tile([128, 3, S], BF, tag="kt")
    v_sb = vpool.tile([128, 16, HPC * 65], BF, tag="v")  # V with ones col per head

    # --- phase 1: QKV projections (x and Wq/Wk/Wv freed afterwards) ---
    with tc.tile_pool(name="xpool", bufs=1) as xpool:
        x_sb = xpool.tile([128, 6, S], BF, tag="x")
        wq_sb = xpool.tile([128, 6, JW], BF, tag="wq")
        wk_sb = xpool.tile([128, 6, JW], BF, tag="wk")
        wv_sb = xpool.tile([128, 6, JW], BF, tag="wv")
        nc.sync.dma_start(x_sb[:], xT.rearrange("(o p) s -> p o s", p=128))
        nc.sync.dma_start(wq_sb[:], wqT.rearrange("(o p) j -> p o j", p=128))
        nc.sync.dma_start(wk_sb[:], wkT.rearrange("(o p) j -> p o j", p=128))
        nc.sync.dma_start(wv_sb[:], wvT.rearrange("(o p) j -> p o j", p=128))

        for w_sb, dest, bcol0 in ((wq_sb, qt_sb, 0), (wk_sb, kt_sb, 3)):
            for jo in range(3):
                for st in range(4):
                    ps = ps_work.tile([128, 512], F32, tag="psw")
                    for o in range(6):
                        nc.tensor.matmul(
                            ps[:],
                            lhsT=w_sb[:, o, ts(jo, 128)],
                            rhs=x_sb[:, o, ts(st, 512)],
                            start=(o == 0),
                            stop=(o == 5),
                        )
                    # evacuate + add per-partition bias (torch Linear bias)
                    nc.vector.tensor_tensor(
                        dest[:, jo, ts(st, 512)], ps[:],
                        bqk_sb[:, bcol0 + jo : bcol0 + jo + 1]
                        .to_broadcast([128, 512]),
                        mybir.AluOpType.add,
                    )

        # ones columns for the softmax-denominator trick
        v4 = v_sb.rearrange("p k (h c) -> p k h c", c=65)
        nc.gpsimd.memset(v4[:, :, :, 64:65], 1.0)
        for kt in range(16):
            ps = ps_work.tile([128, 512], F32, tag="psw")
            for o in range(6):
                nc.tensor.matmul(
                    ps[:, 0:JW],
                    lhsT=x_sb[:, o, ts(kt, 128)],
                    rhs=wv_sb[:, o, :],
                    start=(o == 0),
                    stop=(o == 5),
                )
            # V bias is folded out: context of unnormalized weights plus the
            # colsum row lets the host add bv @ Wo.T once per batch.
            nc.vector.tensor_copy(
                out=v4[:, kt, :, 0:64],
                in_=ps[:, 0:JW].rearrange("p (h c) -> p h c", c=64),
            )

    # --- phase 2: attention, 3-stage pipeline over (head, q-half) units ---
    et_pool = ctx.enter_context(tc.tile_pool(name="eT", bufs=ET_BUFS))
    rp_pool = ctx.enter_context(tc.tile_pool(name="rp", bufs=2))
    cr_pool = ctx.enter_context(tc.tile_pool(name="ctxraw", bufs=2))
    cs_pool = ctx.enter_context(tc.tile_pool(name="csmall", bufs=2))
    ctxT_sb = ctx_pool.tile([128, 3, S], BF, tag="ctxT")
    et_tiles = {}
    rp_tiles = {}
    craw_tiles = {}
    for p in range(NP + 2):
        pu = p if p < NP else None               # scores+exp unit
        pc = p - 1 if 1 <= p <= NP else None     # context unit
        pn = p - 2 if p >= 2 else None           # normalize+ship unit

        if pc is not None:
            craw = cr_pool.tile([65, HQ], F32, tag="craw")
            craw_tiles[pc] = craw
            cs_dram = dram.tile([HQ], F32, tag="cs")
            rb_dram = dram.tile([HQ], BF, tag="rb")
            colsum2 = cs_pool.tile([8, 128], F32, tag="cs2")
            recip2 = cs_pool.tile([8, 128], F32, tag="rc2")
            recipb = cs_pool.tile([8, 128], BF, tag="rcb")
            rp = rp_pool.tile([128, HQ], BF, tag="rp")
            rp_tiles[pc] = rp
            ps_ctx = None

        for kt in range(16):
            if pu is not None:
                h, a = pu // 2, pu % 2
                po, o = 64 * (h % 2), h // 2
                et = et_pool.tile([128, HQ], BF, tag="et")
                et_tiles[(pu, kt)] = et
                ps_s = ps_scores.tile([128, HQ], F32, tag="pss")
                for st in range(2):
                    nc.tensor.matmul(
                        ps_s[:, ts(st, 512)],
                        lhsT=kt_sb[po : po + 64, o, ts(kt, 128)],
                        rhs=qt_sb[po : po + 64, o, ds(HQ * a + 512 * st, 512)],
                        start=True,
                        stop=True,
                    )
                nc.scalar.activation(et[:], ps_s[:], Exp, scale=SCALE)

            if pc is not None:
                hc = pc // 2
                sl, base = kt // 8, 2 * (kt % 8)
                if base == 0:
                    ps_ctx = ps_work.tile([128, 512], F32, tag="psw")
                for k2 in (base, base + 1):
                    nc.tensor.matmul(
                        ps_ctx[0:65, :],
                        lhsT=v_sb[:, k2, ds(65 * hc, 65)],
                        rhs=et_tiles[(pc, k2)][:, ts(sl, 512)],
                        start=(k2 == 0),
                        stop=(k2 == 15),
                    )
                if base == 14:
                    # strip done: rows 0..63 = raw ctx.T, row 64 = colsum.
                    # Evacuate on ScalarE (DVE carries the normalizes).
                    nc.scalar.copy(craw[:, ts(sl, 512)], ps_ctx[0:65, :])
                    nc.sync.dma_start(cs_dram[ts(sl, 512)], craw[64:65, ts(sl, 512)])
                if kt == 15:
                    # colsum -> (8,128) partition-major -> 1/x -> bf16 ->
                    # replicate to 128 partitions via stride-0 DMA
                    nc.sync.dma_start(
                        colsum2[:], cs_dram.rearrange("(a b) -> a b", b=128)
                    )
                    nc.vector.reciprocal(recip2[:], colsum2[:])
                    nc.vector.tensor_copy(recipb[:], recip2[:])
                    nc.sync.dma_start(rb_dram[:], recipb[:])
                    nc.sync.dma_start(
                        rp[:],
                        rb_dram.rearrange("(o f) -> o f", o=1)
                        .to_broadcast([128, HQ]),
                    )

            if pn is not None:
                hn, an = pn // 2, pn % 2
                if kt == 0:
                    # normalize this unit's context rows (raw ctx * recip)
                    pp, oc = 64 * (hn % 2), hn // 2
                    nc.vector.tensor_tensor(
                        ctxT_sb[pp : pp + 64, oc, ds(HQ * an, HQ)],
                        craw_tiles.pop(pn)[0:64, :],
                        rp_tiles[pn][0:64, :],
                        mult,
                    )
                # normalize exp-scores in place and ship, one tile per slot
                et = et_tiles.pop((pn, kt))
                nc.vector.tensor_tensor(et[:], et[:], rp_tiles[pn][:], mult)
                nc.sync.dma_start(
                    w_out[hn, ts(kt, 128), ds(HQ * an, HQ)], et[:]
                )
                if kt == 15:
                    rp_tiles.pop(pn)

    # --- phase 3: output projection (partial; host sums across 2 cores) ---
    with tc.tile_pool(name="outsb", bufs=3) as outp:
        for st2 in range(16):
            ob = outp.tile([128, D], BF, tag="ob")
            for c0, cw in ((0, 512), (512, 256)):
                ps = ps_work.tile([128, 512], F32, tag="psw")
                for o in range(3):
                    nc.tensor.matmul(
                        ps[:, 0:cw],
                        lhsT=ctxT_sb[:, o, ts(st2, 128)],
                        rhs=wo_sb[:, o, ds(c0, cw)],
                        start=(o == 0),
                        stop=(o == 2),
                    )
                nc.scalar.copy(ob[:, ds(c0, cw)], ps[:, 0:cw])
            nc.sync.dma_start(y_out[ts(st2, 128), :], ob[:])


def build_bass(finalize=True):
    nc = bacc.Bacc(None, target_bir_lowering=False, debug=False)
    xT = nc.declare_dram_parameter("xT", [D, S], BF, isOutput=False)
    wqT = nc.declare_dram_parameter("wqT", [D, JW], BF, isOutput=False)
    wkT = nc.declare_dram_parameter("wkT", [D, JW], BF, isOutput=False)
    wvT = nc.declare_dram_parameter("wvT", [D, JW], BF, isOutput=False)
    woT = nc.declare_dram_parameter("woT", [JW, D], BF, isOutput=False)
    bqk = nc.declare_dram_parameter("bqk", [128, 6], F32, isOutput=False)
    w_out = nc.declare_dram_parameter("w_out", [HPC, S, S], BF, isOutput=True)
    y_out = nc.declare_dram_parameter("y_out", [S, D], BF, isOutput=True)

    with tile.TileContext(nc) as tc:
        with ExitStack() as ctx:
            _body(ctx, tc, xT, wqT, wkT, wvT, woT, bqk, w_out, y_out)
    if finalize:
        nc.finalize()
    return nc


@functools.lru_cache(maxsize=1)
def _built():
    return build_bass()


def make_in_maps(x, Wq, bq, Wk, bk, Wv, bv, Wo, bo):
    in_maps = []
    for core in range(NCORES):
        b = core // 2
        hb = HPC * (core % 2)
        rows = slice(hb * DK, hb * DK + JW)
        bqk = np.empty((128, 6), np.float32)
        for jo in range(3):
            bqk[:, jo] = bq[rows][jo * 128 : (jo + 1) * 128]
            bqk[:, 3 + jo] = bk[rows][jo * 128 : (jo + 1) * 128]
        in_maps.append(
            dict(
                xT=np.ascontiguousarray(x[b].T).astype(BF_NP),
                wqT=np.ascontiguousarray(Wq[rows].T).astype(BF_NP),
                wkT=np.ascontiguousarray(Wk[rows].T).astype(BF_NP),
                wvT=np.ascontiguousarray(Wv[rows].T).astype(BF_NP),
                woT=np.ascontiguousarray(Wo[:, rows].T).astype(BF_NP),
                bqk=bqk,
            )
        )
    return in_maps


def assemble(results, Wq=None, bq=None, Wk=None, bk=None, Wv=None, bv=None,
             Wo=None, bo=None):
    """Gather per-core outputs into full (output, weights)."""
    wf = np.empty((B, NH, S, S), np.float32)  # [b, h, k, q] order
    y = np.empty((B, S, D), np.float32)
    const = (Wo @ bv + bo).astype(np.float32)  # bv @ Wo.T + bo
    for b in range(B):
        r0, r1 = results[2 * b], results[2 * b + 1]
        y[b] = (
            np.asarray(r0["y_out"]).astype(np.float32)
            + np.asarray(r1["y_out"]).astype(np.float32)
            + const
        )
        wf[b, 0:HPC] = np.asarray(r0["w_out"]).astype(np.float32)
        wf[b, HPC:NH] = np.asarray(r1["w_out"]).astype(np.float32)
    weights = wf.transpose(0, 1, 3, 2)  # view: [b, h, q, k]
    return y, weights


def kernel(x, Wq, bq, Wk, bk, Wv, bv, Wo, bo):
    from concourse.bass_utils import run_bass_kernel_spmd

    nc = _built()
    in_maps = make_in_maps(x, Wq, bq, Wk, bk, Wv, bv, Wo, bo)
    res = run_bass_kernel_spmd(nc, in_maps, core_ids=list(range(NCORES)))
    return assemble(res.results, Wq, bq, Wk, bk, Wv, bv, Wo, bo)


# revision 17
# speedup vs baseline: 1.4392x; 1.4179x over previous
"""Multi-head attention (B=4, S=2048, D=768, H=12) on 8 trn2 NeuronCores.

Sharding: 48 (batch, head) slices over 8 cores; core i handles batch i//2 and
heads 6*(i%2) .. 6*(i%2)+6 (tensor-parallel columns of Wq/Wk/Wv, rows of Wo).
Host sums the two partial output projections per batch (the "all-reduce after
W_o") and adds the bias terms.

Device-side layout: everything is computed with the contraction dim on SBUF
partitions so no on-chip transposes are needed:
  - host ships x[b].T (D, S) and W*.T slices
  - QT/KT are (384, S) with head-dim on partitions
  - scores are built transposed: ST[k, q] = sum_d KT[d,k] QT[d,q]
  - exp via ScalarE (scale=1/sqrt(dk) folded in); softmax denominators come
    from a ones-column appended to V during the context matmul (row 64 of the
    context PSUM accumulator is the per-query colsum of exp scores)
  - reciprocal of the colsum is computed on a partition-major view (cheap on
    DVE) and replicated to all 128 partitions with a stride-0 DMA
  - attention weights output is written transposed [h][k][q]; the host
    returns a transposed view so no device transpose is ever done.

The attention loop is a 3-stage software pipeline over (head, query-half)
units: phase p runs scores+exp for unit p, context matmuls for unit p-1, and
the softmax-normalize + DMA-out for unit p-2. Query-halving keeps the live
exp-score working set at ~2 generations x 16 x (128,1024)bf16 = 8MB so the
TensorEngine never starves on pool slots (starvation de-warms the PE clock).
"""

import sys
import functools
from contextlib import ExitStack

import numpy as np

if "/opt/trn_rl_repo" not in sys.path:
    sys.path.insert(0, "/opt/trn_rl_repo")

import ml_dtypes  # noqa: E402

import concourse.bass as bass  # noqa: E402
import concourse.mybir as mybir  # noqa: E402
import concourse.tile as tile  # noqa: E402
from concourse import bacc  # noqa: E402
from concourse.bass import ts, ds  # noqa: E402

BF = mybir.dt.bfloat16
F32 = mybir.dt.float32
BF_NP = ml_dtypes.bfloat16

D = 768          # model dim
S = 2048         # sequence length
NH = 12          # total heads
DK = 64          # head dim
B = 4            # batch
HPC = 6          # heads per core
JW = HPC * DK    # 384: local head width per core
NCORES = 8
SCALE = 1.0 / 8.0  # 1/sqrt(DK)
HQ = 1024        # query-half width
NP = 12          # pipeline units: HPC heads x 2 query halves
ET_BUFS = 50     # eT pool: (128,1024)bf16 tiles, ~2KB/partition each
                 # (3 live generations x 16 + slack so the PE never starves)


def _body(ctx, tc, xT, wqT, wkT, wvT, woT, bqk, w_out, y_out):
    nc = tc.nc
    Exp = mybir.ActivationFunctionType.Exp
    mult = mybir.AluOpType.mult

    consts = ctx.enter_context(tc.tile_pool(name="consts", bufs=1))
    wpool = ctx.enter_context(tc.tile_pool(name="wpool", bufs=1))
    qk_pool = ctx.enter_context(tc.tile_pool(name="qk", bufs=1))
    vpool = ctx.enter_context(tc.tile_pool(name="vaug", bufs=1))
    ctx_pool = ctx.enter_context(tc.tile_pool(name="ctxsb", bufs=1))
    dram = ctx.enter_context(tc.tile_pool(name="dram", bufs=3, space="DRAM"))
    ps_scores = ctx.enter_context(tc.tile_pool(name="ps_s", bufs=3, space="PSUM"))
    ps_work = ctx.enter_context(tc.tile_pool(name="ps_w", bufs=2, space="PSUM"))

    wo_sb = wpool.tile([128, 3, D], BF, tag="wo")
    bqk_sb = consts.tile([128, 6], F32, tag="bqk")
    nc.sync.dma_start(wo_sb[:], woT.rearrange("(o p) e -> p o e", p=128))
    nc.sync.dma_start(bqk_sb[:], bqk[:, :])

    qt_sb = qk_pool.tile([128, 3, S], BF, tag="qt")
    kt_sb = qk_pool.tile([128, 3, S], BF, tag="kt")
    v_sb = vpool.tile([128, 16, HPC * 65], BF, tag="v")  # V with ones col per head

    # --- phase 1: QKV projections (x and Wq/Wk/Wv freed afterwards) ---
    with tc.tile_pool(name="xpool", bufs=1) as xpool:
        x_sb = xpool.tile([128, 6, S], BF, tag="x")
        wq_sb = xpool.tile([128, 6, JW], BF, tag="wq")
        wk_sb = xpool.tile([128, 6, JW], BF, tag="wk")
        wv_sb = xpool.tile([128, 6, JW], BF, tag="wv")
        nc.sync.dma_start(x_sb[:], xT.rearrange("(o p) s -> p o s", p=128))
        nc.sync.dma_start(wq_sb[:], wqT.rearrange("(o p) j -> p o j", p=128))
        nc.sync.dma_start(wk_sb[:], wkT.rearrange("(o p) j -> p o j", p=128))
        nc.sync.dma_start(wv_sb[:], wvT.rearrange("(o p) j -> p o j", p=128))

        for w_sb, dest, bcol0 in ((wq_sb, qt_sb, 0), (wk_sb, kt_sb, 3)):
            for jo in range(3):
                for st in range(4):
                    ps = ps_work.tile([128, 512], F32, tag="psw")
                    for o in range(6):
                        nc.tensor.matmul(
                            ps[:],
                            lhsT=w_sb[:, o, ts(jo, 128)],
                            rhs=x_sb[:, o, ts(st, 512)],
                            start=(o == 0),
                            stop=(o == 5),
                        )
                    # evacuate + add per-partition bias (torch Linear bias).
                    # ScalarE is idle during this phase; keep DVE free.
                    nc.scalar.activation(
                        dest[:, jo, ts(st, 512)], ps[:],
                        mybir.ActivationFunctionType.Identity,
                        bias=bqk_sb[:, bcol0 + jo : bcol0 + jo + 1],
                    )

        # ones columns for the softmax-denominator trick
        v4 = v_sb.rearrange("p k (h c) -> p k h c", c=65)
        nc.gpsimd.memset(v4[:, :, :, 64:65], 1.0)
        for kt in range(16):
            ps = ps_work.tile([128, 512], F32, tag="psw")
            for o in range(6):
                nc.tensor.matmul(
                    ps[:, 0:JW],
                    lhsT=x_sb[:, o, ts(kt, 128)],
                    rhs=wv_sb[:, o, :],
                    start=(o == 0),
                    stop=(o == 5),
                )
            # V bias is folded out: context of unnormalized weights plus the
            # colsum row lets the host add bv @ Wo.T once per batch.
            nc.scalar.copy(
                v4[:, kt, :, 0:64],
                ps[:, 0:JW].rearrange("p (h c) -> p h c", c=64),
            )

    # --- phase 2: attention, 3-stage pipeline over (head, q-half) units ---
    et_pool = ctx.enter_context(tc.tile_pool(name="eT", bufs=ET_BUFS))
    rp_pool = ctx.enter_context(tc.tile_pool(name="rp", bufs=2))
    cr_pool = ctx.enter_context(tc.tile_pool(name="ctxraw", bufs=2))
    cs_pool = ctx.enter_context(tc.tile_pool(name="csmall", bufs=2))
    ctxT_sb = ctx_pool.tile([128, 3, S], BF, tag="ctxT")
    et_tiles = {}
    rp_tiles = {}
    craw_tiles = {}
    for p in range(NP + 2):
        pu = p if p < NP else None               # scores+exp unit
        pc = p - 1 if 1 <= p <= NP else None     # context unit
        pn = p - 2 if p >= 2 else None           # normalize+ship unit

        if pc is not None:
            craw = cr_pool.tile([65, HQ], F32, tag="craw")
            craw_tiles[pc] = craw
            cs_dram = dram.tile([HQ], F32, tag="cs")
            rb_dram = dram.tile([HQ], BF, tag="rb")
            colsum2 = cs_pool.tile([8, 128], F32, tag="cs2")
            recip2 = cs_pool.tile([8, 128], F32, tag="rc2")
            recipb = cs_pool.tile([8, 128], BF, tag="rcb")
            rp = rp_pool.tile([128, HQ], BF, tag="rp")
            rp_tiles[pc] = rp
            ps_ctx = None

        for kt in range(16):
            if pu is not None:
                h, a = pu // 2, pu % 2
                po, o = 64 * (h % 2), h // 2
                et = et_pool.tile([128, HQ], BF, tag="et")
                et_tiles[(pu, kt)] = et
                ps_s = ps_scores.tile([128, HQ], F32, tag="pss")
                for st in range(2):
                    nc.tensor.matmul(
                        ps_s[:, ts(st, 512)],
                        lhsT=kt_sb[po : po + 64, o, ts(kt, 128)],
                        rhs=qt_sb[po : po + 64, o, ds(HQ * a + 512 * st, 512)],
                        start=True,
                        stop=True,
                    )
                nc.scalar.activation(et[:], ps_s[:], Exp, scale=SCALE)

            if pc is not None:
                hc = pc // 2
                sl, base = kt // 8, 2 * (kt % 8)
                if base == 0:
                    ps_ctx = ps_work.tile([128, 512], F32, tag="psw")
                for k2 in (base, base + 1):
                    nc.tensor.matmul(
                        ps_ctx[0:65, :],
                        lhsT=v_sb[:, k2, ds(65 * hc, 65)],
                        rhs=et_tiles[(pc, k2)][:, ts(sl, 512)],
                        start=(k2 == 0),
                        stop=(k2 == 15),
                    )
                if base == 14:
                    # strip done: rows 0..63 = raw ctx.T, row 64 = colsum.
                    # Evacuate on DVE (ScalarE is saturated with exp).
                    nc.vector.tensor_copy(craw[:, ts(sl, 512)], ps_ctx[0:65, :])
                    nc.sync.dma_start(cs_dram[ts(sl, 512)], craw[64:65, ts(sl, 512)])
                if kt == 15:
                    # colsum -> (8,128) partition-major -> 1/x -> bf16 ->
                    # replicate to 128 partitions via stride-0 DMA
                    nc.sync.dma_start(
                        colsum2[:], cs_dram.rearrange("(a b) -> a b", b=128)
                    )
                    nc.vector.reciprocal(recip2[:], colsum2[:])
                    nc.vector.tensor_copy(recipb[:], recip2[:])
                    nc.sync.dma_start(rb_dram[:], recipb[:])
                    nc.sync.dma_start(
                        rp[:],
                        rb_dram.rearrange("(o f) -> o f", o=1)
                        .to_broadcast([128, HQ]),
                    )

            if pn is not None:
                hn, an = pn // 2, pn % 2
                if kt == 0:
                    # normalize this unit's context rows (raw ctx * recip)
                    pp, oc = 64 * (hn % 2), hn // 2
                    nc.vector.tensor_tensor(
                        ctxT_sb[pp : pp + 64, oc, ds(HQ * an, HQ)],
                        craw_tiles.pop(pn)[0:64, :],
                        rp_tiles[pn][0:64, :],
                        mult,
                    )
                # normalize exp-scores in place and ship, one tile per slot
                et = et_tiles.pop((pn, kt))
                nc.vector.tensor_tensor(et[:], et[:], rp_tiles[pn][:], mult)
                nc.sync.dma_start(
                    w_out[hn, ts(kt, 128), ds(HQ * an, HQ)], et[:]
                )
                if kt == 15:
                    rp_tiles.pop(pn)

    # --- phase 3: output projection (partial; host sums across 2 cores) ---
    with tc.tile_pool(name="outsb", bufs=3) as outp:
        for st2 in range(16):
            ob = outp.tile([128, D], BF, tag="ob")
            for c0, cw in ((0, 512), (512, 256)):
                ps = ps_work.tile([128, 512], F32, tag="psw")
                for o in range(3):
                    nc.tensor.matmul(
                        ps[:, 0:cw],
                        lhsT=ctxT_sb[:, o, ts(st2, 128)],
                        rhs=wo_sb[:, o, ds(c0, cw)],
                        start=(o == 0),
                        stop=(o == 2),
                    )
                nc.scalar.copy(ob[:, ds(c0, cw)], ps[:, 0:cw])
            nc.sync.dma_start(y_out[ts(st2, 128), :], ob[:])


def build_bass(finalize=True):
    nc = bacc.Bacc(None, target_bir_lowering=False, debug=False)
    xT = nc.declare_dram_parameter("xT", [D, S], BF, isOutput=False)
    wqT = nc.declare_dram_parameter("wqT", [D, JW], BF, isOutput=False)
    wkT = nc.declare_dram_parameter("wkT", [D, JW], BF, isOutput=False)
    wvT = nc.declare_dram_parameter("wvT", [D, JW], BF, isOutput=False)
    woT = nc.declare_dram_parameter("woT", [JW, D], BF, isOutput=False)
    bqk = nc.declare_dram_parameter("bqk", [128, 6], F32, isOutput=False)
    w_out = nc.declare_dram_parameter("w_out", [HPC, S, S], BF, isOutput=True)
    y_out = nc.declare_dram_parameter("y_out", [S, D], BF, isOutput=True)

    with tile.TileContext(nc) as tc:
        with ExitStack() as ctx:
            _body(ctx, tc, xT, wqT, wkT, wvT, woT, bqk, w_out, y_out)
    if finalize:
        nc.finalize()
    return nc


@functools.lru_cache(maxsize=1)
def _built():
    return build_bass()


def make_in_maps(x, Wq, bq, Wk, bk, Wv, bv, Wo, bo):
    in_maps = []
    for core in range(NCORES):
        b = core // 2
        hb = HPC * (core % 2)
        rows = slice(hb * DK, hb * DK + JW)
        bqk = np.empty((128, 6), np.float32)
        for jo in range(3):
            bqk[:, jo] = bq[rows][jo * 128 : (jo + 1) * 128]
            bqk[:, 3 + jo] = bk[rows][jo * 128 : (jo + 1) * 128]
        in_maps.append(
            dict(
                xT=np.ascontiguousarray(x[b].T).astype(BF_NP),
                wqT=np.ascontiguousarray(Wq[rows].T).astype(BF_NP),
                wkT=np.ascontiguousarray(Wk[rows].T).astype(BF_NP),
                wvT=np.ascontiguousarray(Wv[rows].T).astype(BF_NP),
                woT=np.ascontiguousarray(Wo[:, rows].T).astype(BF_NP),
                bqk=bqk,
            )
        )
    return in_maps


def assemble(results, Wq=None, bq=None, Wk=None, bk=None, Wv=None, bv=None,
             Wo=None, bo=None):
    """Gather per-core outputs into full (output, weights)."""
    wf = np.empty((B, NH, S, S), np.float32)  # [b, h, k, q] order
    y = np.empty((B, S, D), np.float32)
    const = (Wo @ bv + bo).astype(np.float32)  # bv @ Wo.T + bo
    for b in range(B):
        r0, r1 = results[2 * b], results[2 * b + 1]
        y[b] = (
            np.asarray(r0["y_out"]).astype(np.float32)
            + np.asarray(r1["y_out"]).astype(np.float32)
            + const
        )
        wf[b, 0:HPC] = np.asarray(r0["w_out"]).astype(np.float32)
        wf[b, HPC:NH] = np.asarray(r1["w_out"]).astype(np.float32)
    weights = wf.transpose(0, 1, 3, 2)  # view: [b, h, q, k]
    return y, weights


def kernel(x, Wq, bq, Wk, bk, Wv, bv, Wo, bo):
    from concourse.bass_utils import run_bass_kernel_spmd

    nc = _built()
    in_maps = make_in_maps(x, Wq, bq, Wk, bk, Wv, bv, Wo, bo)
    res = run_bass_kernel_spmd(nc, in_maps, core_ids=list(range(NCORES)))
    return assemble(res.results, Wq, bq, Wk, bk, Wv, bv, Wo, bo)


# revision 20
# speedup vs baseline: 1.4716x; 1.0226x over previous
"""Multi-head attention (B=4, S=2048, D=768, H=12) on 8 trn2 NeuronCores.

Sharding: 48 (batch, head) slices over 8 cores; core i handles batch i//2 and
heads 6*(i%2) .. 6*(i%2)+6 (tensor-parallel columns of Wq/Wk/Wv, rows of Wo).
Host sums the two partial output projections per batch (the "all-reduce after
W_o") and adds the bias terms.

Device-side layout: everything is computed with the contraction dim on SBUF
partitions so no on-chip transposes are needed:
  - host ships x[b].T (D, S) and W*.T slices
  - QT/KT are (384, S) with head-dim on partitions
  - scores are built transposed: ST[k, q] = sum_d KT[d,k] QT[d,q]
  - exp via ScalarE (scale=1/sqrt(dk) folded in); softmax denominators come
    from a ones-column appended to V during the context matmul (row 64 of the
    context PSUM accumulator is the per-query colsum of exp scores)
  - reciprocal of the colsum is computed on a partition-major view (cheap on
    DVE) and replicated to all 128 partitions with a stride-0 DMA
  - attention weights output is written transposed [h][k][q]; the host
    returns a transposed view so no device transpose is ever done.

The attention loop is a 3-stage software pipeline over (head, query-half)
units: phase p runs scores+exp for unit p, context matmuls for unit p-1, and
the softmax-normalize + DMA-out for unit p-2. Query-halving keeps the live
exp-score working set at ~2 generations x 16 x (128,1024)bf16 = 8MB so the
TensorEngine never starves on pool slots (starvation de-warms the PE clock).
"""

import sys
import functools
from contextlib import ExitStack

import numpy as np

if "/opt/trn_rl_repo" not in sys.path:
    sys.path.insert(0, "/opt/trn_rl_repo")

import ml_dtypes  # noqa: E402

import concourse.bass as bass  # noqa: E402
import concourse.mybir as mybir  # noqa: E402
import concourse.tile as tile  # noqa: E402
from concourse import bacc  # noqa: E402
from concourse.bass import ts, ds  # noqa: E402

BF = mybir.dt.bfloat16
F32 = mybir.dt.float32
BF_NP = ml_dtypes.bfloat16

D = 768          # model dim
S = 2048         # sequence length
NH = 12          # total heads
DK = 64          # head dim
B = 4            # batch
HPC = 6          # heads per core
JW = HPC * DK    # 384: local head width per core
NCORES = 8
SCALE = 1.0 / 8.0  # 1/sqrt(DK)
HQ = 1024        # query-half width
NP = 12          # pipeline units: HPC heads x 2 query halves
ET_BUFS = 50     # eT pool: (128,1024)bf16 tiles, ~2KB/partition each
                 # (3 live generations x 16 + slack so the PE never starves)


def _body(ctx, tc, xT, wqT, wkT, wvT, woT, bqk, w_out, y_out):
    nc = tc.nc
    Exp = mybir.ActivationFunctionType.Exp
    mult = mybir.AluOpType.mult

    consts = ctx.enter_context(tc.tile_pool(name="consts", bufs=1))
    wpool = ctx.enter_context(tc.tile_pool(name="wpool", bufs=1))
    qk_pool = ctx.enter_context(tc.tile_pool(name="qk", bufs=1))
    vpool = ctx.enter_context(tc.tile_pool(name="vaug", bufs=1))
    ctx_pool = ctx.enter_context(tc.tile_pool(name="ctxsb", bufs=1))
    dram = ctx.enter_context(tc.tile_pool(name="dram", bufs=3, space="DRAM"))
    ps_scores = ctx.enter_context(tc.tile_pool(name="ps_s", bufs=3, space="PSUM"))
    ps_work = ctx.enter_context(tc.tile_pool(name="ps_w", bufs=2, space="PSUM"))

    wo_sb = wpool.tile([128, 3, D], BF, tag="wo")
    bqk_sb = consts.tile([128, 6], F32, tag="bqk")
    nc.sync.dma_start(wo_sb[:], woT.rearrange("(o p) e -> p o e", p=128))
    nc.sync.dma_start(bqk_sb[:], bqk[:, :])

    qt_sb = qk_pool.tile([128, 3, S], BF, tag="qt")
    kt_sb = qk_pool.tile([128, 3, S], BF, tag="kt")
    v_sb = vpool.tile([128, 16, HPC * 65], BF, tag="v")  # V with ones col per head

    # --- phase 1: QKV projections (x and Wq/Wk/Wv freed afterwards) ---
    with tc.tile_pool(name="xpool", bufs=1) as xpool:
        x_sb = xpool.tile([128, 6, S], BF, tag="x")
        wq_sb = xpool.tile([128, 6, JW], BF, tag="wq")
        wk_sb = xpool.tile([128, 6, JW], BF, tag="wk")
        wv_sb = xpool.tile([128, 6, JW], BF, tag="wv")
        nc.sync.dma_start(x_sb[:], xT.rearrange("(o p) s -> p o s", p=128))
        nc.sync.dma_start(wq_sb[:], wqT.rearrange("(o p) j -> p o j", p=128))
        nc.sync.dma_start(wk_sb[:], wkT.rearrange("(o p) j -> p o j", p=128))
        nc.sync.dma_start(wv_sb[:], wvT.rearrange("(o p) j -> p o j", p=128))

        for w_sb, dest, bcol0 in ((wq_sb, qt_sb, 0), (wk_sb, kt_sb, 3)):
            for jo in range(3):
                for st in range(4):
                    ps = ps_work.tile([128, 512], F32, tag="psw")
                    for o in range(6):
                        nc.tensor.matmul(
                            ps[:],
                            lhsT=w_sb[:, o, ts(jo, 128)],
                            rhs=x_sb[:, o, ts(st, 512)],
                            start=(o == 0),
                            stop=(o == 5),
                        )
                    # evacuate + add per-partition bias (torch Linear bias).
                    # ScalarE is idle during this phase; keep DVE free.
                    nc.scalar.activation(
                        dest[:, jo, ts(st, 512)], ps[:],
                        mybir.ActivationFunctionType.Identity,
                        bias=bqk_sb[:, bcol0 + jo : bcol0 + jo + 1],
                    )

        # ones columns for the softmax-denominator trick
        v4 = v_sb.rearrange("p k (h c) -> p k h c", c=65)
        nc.gpsimd.memset(v4[:, :, :, 64:65], 1.0)
        for kt in range(16):
            ps = ps_work.tile([128, 512], F32, tag="psw")
            for o in range(6):
                nc.tensor.matmul(
                    ps[:, 0:JW],
                    lhsT=x_sb[:, o, ts(kt, 128)],
                    rhs=wv_sb[:, o, :],
                    start=(o == 0),
                    stop=(o == 5),
                )
            # V bias is folded out: context of unnormalized weights plus the
            # colsum row lets the host add bv @ Wo.T once per batch.
            nc.scalar.copy(
                v4[:, kt, :, 0:64],
                ps[:, 0:JW].rearrange("p (h c) -> p h c", c=64),
            )

    # --- phase 2: attention, 3-stage pipeline over (head, q-half) units ---
    et_pool = ctx.enter_context(tc.tile_pool(name="eT", bufs=ET_BUFS))
    outp = ctx.enter_context(tc.tile_pool(name="outsb", bufs=3))
    rp_pool = ctx.enter_context(tc.tile_pool(name="rp", bufs=2))
    cr_pool = ctx.enter_context(tc.tile_pool(name="ctxraw", bufs=2))
    cs_pool = ctx.enter_context(tc.tile_pool(name="csmall", bufs=2))
    ctxT_sb = ctx_pool.tile([128, 3, S], BF, tag="ctxT")
    et_tiles = {}
    rp_tiles = {}
    craw_tiles = {}
    for p in range(NP + 2):
        pu = p if p < NP else None               # scores+exp unit
        pc = p - 1 if 1 <= p <= NP else None     # context unit
        pn = p - 2 if p >= 2 else None           # normalize+ship unit

        if pc is not None:
            craw = cr_pool.tile([65, HQ], F32, tag="craw")
            craw_tiles[pc] = craw
            cs_dram = dram.tile([HQ], F32, tag="cs")
            rb_dram = dram.tile([HQ], BF, tag="rb")
            colsum2 = cs_pool.tile([8, 128], F32, tag="cs2")
            recip2 = cs_pool.tile([8, 128], F32, tag="rc2")
            recipb = cs_pool.tile([8, 128], BF, tag="rcb")
            rp = rp_pool.tile([128, HQ], BF, tag="rp")
            rp_tiles[pc] = rp
            ps_ctx = None

        for kt in range(16):
            if pu is not None:
                h, a = pu // 2, pu % 2
                po, o = 64 * (h % 2), h // 2
                et = et_pool.tile([128, HQ], BF, tag="et")
                et_tiles[(pu, kt)] = et
                ps_s = ps_scores.tile([128, HQ], F32, tag="pss")
                for st in range(2):
                    nc.tensor.matmul(
                        ps_s[:, ts(st, 512)],
                        lhsT=kt_sb[po : po + 64, o, ts(kt, 128)],
                        rhs=qt_sb[po : po + 64, o, ds(HQ * a + 512 * st, 512)],
                        start=True,
                        stop=True,
                    )
                nc.scalar.activation(et[:], ps_s[:], Exp, scale=SCALE)

            if pc is not None:
                hc = pc // 2
                sl, base = kt // 8, 2 * (kt % 8)
                if base == 0:
                    ps_ctx = ps_work.tile([128, 512], F32, tag="psw")
                for k2 in (base, base + 1):
                    nc.tensor.matmul(
                        ps_ctx[0:65, :],
                        lhsT=v_sb[:, k2, ds(65 * hc, 65)],
                        rhs=et_tiles[(pc, k2)][:, ts(sl, 512)],
                        start=(k2 == 0),
                        stop=(k2 == 15),
                    )
                if base == 14:
                    # strip done: rows 0..63 = raw ctx.T, row 64 = colsum.
                    # Evacuate on DVE (ScalarE is saturated with exp).
                    nc.vector.tensor_copy(craw[:, ts(sl, 512)], ps_ctx[0:65, :])
                    nc.sync.dma_start(cs_dram[ts(sl, 512)], craw[64:65, ts(sl, 512)])
                if kt == 15:
                    # colsum -> (8,128) partition-major -> 1/x -> bf16 ->
                    # replicate to 128 partitions via stride-0 DMA
                    nc.sync.dma_start(
                        colsum2[:], cs_dram.rearrange("(a b) -> a b", b=128)
                    )
                    nc.vector.reciprocal(recip2[:], colsum2[:])
                    nc.vector.tensor_copy(recipb[:], recip2[:])
                    nc.sync.dma_start(rb_dram[:], recipb[:])
                    nc.sync.dma_start(
                        rp[:],
                        rb_dram.rearrange("(o f) -> o f", o=1)
                        .to_broadcast([128, HQ]),
                    )

            if pn is not None:
                hn, an = pn // 2, pn % 2
                if kt == 0:
                    # normalize this unit's context rows (raw ctx * recip)
                    pp, oc = 64 * (hn % 2), hn // 2
                    nc.vector.tensor_tensor(
                        ctxT_sb[pp : pp + 64, oc, ds(HQ * an, HQ)],
                        craw_tiles.pop(pn)[0:64, :],
                        rp_tiles[pn][0:64, :],
                        mult,
                    )
                # normalize exp-scores in place and ship, one tile per slot
                et = et_tiles.pop((pn, kt))
                nc.vector.tensor_tensor(et[:], et[:], rp_tiles[pn][:], mult)
                nc.sync.dma_start(
                    w_out[hn, ts(kt, 128), ds(HQ * an, HQ)], et[:]
                )
                if kt == 15:
                    rp_tiles.pop(pn)

            # output projection interleaved into the two tail phases: the
            # ctxT columns of q-half an become complete once the last head's
            # ctx-norm for that half ran (kt==0 of phase NP+an). The scores
            # PSUM pool is idle in the tail, so proj accumulates there.
            if p >= NP and 1 <= kt <= 8:
                an = p - NP
                st2 = 8 * an + kt - 1
                ob = outp.tile([128, D], BF, tag="ob")
                for c0, cw in ((0, 512), (512, 256)):
                    ps = ps_scores.tile([128, HQ], F32, tag="pss")
                    for o in range(3):
                        nc.tensor.matmul(
                            ps[:, 0:cw],
                            lhsT=ctxT_sb[:, o, ts(st2, 128)],
                            rhs=wo_sb[:, o, ds(c0, cw)],
                            start=(o == 0),
                            stop=(o == 2),
                        )
                    nc.scalar.copy(ob[:, ds(c0, cw)], ps[:, 0:cw])
                nc.sync.dma_start(y_out[ts(st2, 128), :], ob[:])


def build_bass(finalize=True):
    nc = bacc.Bacc(None, target_bir_lowering=False, debug=False)
    xT = nc.declare_dram_parameter("xT", [D, S], BF, isOutput=False)
    wqT = nc.declare_dram_parameter("wqT", [D, JW], BF, isOutput=False)
    wkT = nc.declare_dram_parameter("wkT", [D, JW], BF, isOutput=False)
    wvT = nc.declare_dram_parameter("wvT", [D, JW], BF, isOutput=False)
    woT = nc.declare_dram_parameter("woT", [JW, D], BF, isOutput=False)
    bqk = nc.declare_dram_parameter("bqk", [128, 6], F32, isOutput=False)
    w_out = nc.declare_dram_parameter("w_out", [HPC, S, S], BF, isOutput=True)
    y_out = nc.declare_dram_parameter("y_out", [S, D], BF, isOutput=True)

    with tile.TileContext(nc) as tc:
        with ExitStack() as ctx:
            _body(ctx, tc, xT, wqT, wkT, wvT, woT, bqk, w_out, y_out)
    if finalize:
        nc.finalize()
    return nc


@functools.lru_cache(maxsize=1)
def _built():
    return build_bass()


def make_in_maps(x, Wq, bq, Wk, bk, Wv, bv, Wo, bo):
    in_maps = []
    for core in range(NCORES):
        b = core // 2
        hb = HPC * (core % 2)
        rows = slice(hb * DK, hb * DK + JW)
        bqk = np.empty((128, 6), np.float32)
        for jo in range(3):
            bqk[:, jo] = bq[rows][jo * 128 : (jo + 1) * 128]
            bqk[:, 3 + jo] = bk[rows][jo * 128 : (jo + 1) * 128]
        in_maps.append(
            dict(
                xT=np.ascontiguousarray(x[b].T).astype(BF_NP),
                wqT=np.ascontiguousarray(Wq[rows].T).astype(BF_NP),
                wkT=np.ascontiguousarray(Wk[rows].T).astype(BF_NP),
                wvT=np.ascontiguousarray(Wv[rows].T).astype(BF_NP),
                woT=np.ascontiguousarray(Wo[:, rows].T).astype(BF_NP),
                bqk=bqk,
            )
        )
    return in_maps


def assemble(results, Wq=None, bq=None, Wk=None, bk=None, Wv=None, bv=None,
             Wo=None, bo=None):
    """Gather per-core outputs into full (output, weights)."""
    wf = np.empty((B, NH, S, S), np.float32)  # [b, h, k, q] order
    y = np.empty((B, S, D), np.float32)
    const = (Wo @ bv + bo).astype(np.float32)  # bv @ Wo.T + bo
    for b in range(B):
        r0, r1 = results[2 * b], results[2 * b + 1]
        y[b] = (
            np.asarray(r0["y_out"]).astype(np.float32)
            + np.asarray(r1["y_out"]).astype(np.float32)
            + const
        )
        wf[b, 0:HPC] = np.asarray(r0["w_out"]).astype(np.float32)
        wf[b, HPC:NH] = np.asarray(r1["w_out"]).astype(np.float32)
    weights = wf.transpose(0, 1, 3, 2)  # view: [b, h, q, k]
    return y, weights


def kernel(x, Wq, bq, Wk, bk, Wv, bv, Wo, bo):
    from concourse.bass_utils import run_bass_kernel_spmd

    nc = _built()
    in_maps = make_in_maps(x, Wq, bq, Wk, bk, Wv, bv, Wo, bo)
    res = run_bass_kernel_spmd(nc, in_maps, core_ids=list(range(NCORES)))
    return assemble(res.results, Wq, bq, Wk, bk, Wv, bv, Wo, bo)
